# revision 1
# baseline (speedup 1.0000x reference)
"""3-layer GAT on 8 trn2 NeuronCores.

Sharding: nodes split 6272/core (dst ownership); edges sharded by SRC so the
f[src] gather is core-local. Each core computes partial dst aggregations for
all 50176 (padded) nodes; 7 pipelined ReduceScatters deliver summed partials
to the dst owner, which normalizes by the softmax denominator z (carried as an
extra column), applies relu (or final log_softmax), and feeds the next layer.

Per-edge softmax numerator exp(leaky_relu(el[src]+er[dst])) is built with no
segment-max: score magnitudes are O(1) so exp is safe in fp32 (softmax is
shift-invariant, so this matches the reference exactly in exact math).

Edge chunks of 128 live one-per-partition. Per chunk a one-hot matrix
Oex[p, d] = (d == dst_local[p]) * ex[p] is built in one DVE tensor_scalar op;
TensorE then computes both the feature aggregation (rhs = gathered f rows) and
z (rhs = ones) with lhsT = Oex accumulated in PSUM per 128-dst window.
er[dst] is fetched by gathering 256B rows of er_full viewed as [*, 64] at row
dst>>6, then a fused scalar_tensor_tensor selects column dst&63 via accum_out.
Layer-1 edge scores are fully host-precomputed (x == h is an input).
"""

import os
import numpy as np

N, E, DIN, DH, DOUT = 50000, 800000, 256, 128, 64
NCORES = 8
PC = 6272            # nodes per core
NPAD = PC * NCORES   # 50176
WPC = 49             # windows (128 dst) per core
NW = WPC * NCORES    # 392
NSEG = 7             # ReduceScatter pipeline segments
WPS = WPC // NSEG    # own windows per segment = 7
GMAX = 24            # max chunks per gather group
P = 128


def _wrap16(idx, dtype=np.int16):
    """dma_gather index layout: [128, n/16]; idx j at [j%16 + 16k, j//16]."""
    n = len(idx)
    out = np.zeros((P, n // 16), dtype=dtype)
    out[:16, :] = idx.astype(dtype).reshape(-1, 16).T
    out[16:, :] = np.tile(out[:16, :], (7, 1))
    return out


def _leaky(x):
    return np.maximum(x, 0.2 * x)


def host_prep(h, src, dst, W1, al1, ar1, W2, al2, ar2, W3, al3, ar3):
    """Numpy-side sharding, edge bucketing, and layer-1 score precompute."""
    f32 = np.float32
    h = np.asarray(h, f32)
    src = np.asarray(src, np.int64)
    dst = np.asarray(dst, np.int64)

    hp = np.zeros((NPAD, DIN), f32)
    hp[:N] = h

    # layer-1 edge scores on host (x == h)
    wl1 = (np.asarray(W1, f32) @ np.asarray(al1, f32)).astype(f32)
    wr1 = (np.asarray(W1, f32) @ np.asarray(ar1, f32)).astype(f32)
    el1 = hp @ wl1
    er1 = hp @ wr1
    ex1_edge = np.exp(_leaky(el1[src] + er1[dst])).astype(f32)

    core_of = src // PC
    win_of = dst >> 7

    # per (core, window) edge counts -> shared chunk schedule
    cnt = np.zeros((NCORES, NW), np.int64)
    np.add.at(cnt, (core_of, win_of), 1)
    nch = np.maximum(1, -(-cnt.max(axis=0) // P))  # >=1 so every window is staged

    # window processing order: segment-major so ReduceScatter s can fire early.
    # window w = cblk*49 + s*7 + j  ->  order (s, cblk, j)
    worder = []
    for s in range(NSEG):
        for cblk in range(NCORES):
            for j in range(WPS):
                worder.append(cblk * WPC + s * WPS + j)
    worder = np.array(worder)

    chunk_win = []           # global window id per chunk, in processing order
    win_chunk0 = np.zeros(NW, np.int64)
    off = 0
    for w in worder:
        win_chunk0[w] = off
        chunk_win.extend([w] * nch[w])
        off += nch[w]
    TOT = off
    chunk_win = np.array(chunk_win)

    # segment boundaries in chunk space + gather groups + window meta
    seg_meta = []            # per segment: list of groups; group = (c0, c1, [(w, stage_row, wc0, wc1)])
    ci = 0
    for s in range(NSEG):
        wins = worder[s * WPS * NCORES:(s + 1) * WPS * NCORES]
        groups = []
        gc0 = ci
        gwins = []
        for k, w in enumerate(wins):
            cblk, j = w // WPC, (w % WPC) % WPS
            stage_row = (cblk * WPS + j) * P
            nw = nch[w]
            if ci + nw - gc0 > GMAX and gwins:
                groups.append((gc0, ci, gwins))
                gc0, gwins = ci, []
            gwins.append((w, stage_row, ci - gc0 + 0, ci - gc0 + nw))
            ci += nw
        groups.append((gc0, ci, gwins))
        seg_meta.append(groups)
    assert ci == TOT

    # per-core slot arrays
    in_maps = []
    for c in range(NCORES):
        sel = np.nonzero(core_of == c)[0]
        e_dst = dst[sel]
        e_src = src[sel]
        e_ex1 = ex1_edge[sel]
        # sort core edges by processing position of their window
        wpos = np.empty(NW, np.int64)
        wpos[worder] = np.arange(NW)
        order = np.argsort(wpos[e_dst >> 7], kind="stable")
        e_dst, e_src, e_ex1 = e_dst[order], e_src[order], e_ex1[order]

        srcl = np.zeros(TOT * P, np.int64)
        dstg = np.zeros(TOT * P, np.int64)
        dstloc = np.full(TOT * P, -1.0, f32)
        exv = np.zeros(TOT * P, f32)

        # slot index for each edge: windows in processing order, chunks dense
        ew = e_dst >> 7
        # position within its window run
        w_seen = np.zeros(NW, np.int64)
        pos_in_w = np.empty(len(sel), np.int64)
        # e_dst sorted by window position -> run-length positions
        uniq, start_idx, counts = np.unique(wpos[ew], return_index=True, return_counts=True)
        for u, st, ct in zip(uniq, start_idx, counts):
            pos_in_w[st:st + ct] = np.arange(ct)
        slot = win_chunk0[ew] * P + pos_in_w
        srcl[slot] = e_src - c * PC
        dstg[slot] = e_dst
        dstloc[slot] = (e_dst & 127).astype(f32)
        exv[slot] = e_ex1
        # pad slots: point dst-row gather at a valid row
        pad = dstloc < 0
        dstg[pad] = 0

        m = dict(
            hT=np.ascontiguousarray(hp[c * PC:(c + 1) * PC].T),
            sidx=_wrap16(srcl),
            dhi=_wrap16(dstg >> 6),
            dlo=np.ascontiguousarray((dstg & 63).astype(f32).reshape(TOT, P).T),
            dstloc=np.ascontiguousarray(dstloc.reshape(TOT, P).T),
            ex1=np.ascontiguousarray(exv.reshape(TOT, P).T),
            iota_t=np.tile(np.arange(P, dtype=f32), (P, 1)),
            ident_t=np.eye(P, dtype=f32),
            W1=np.asarray(W1, f32),
            W2=np.asarray(W2, f32),
            W3=np.asarray(W3, f32),
            wl2=(np.asarray(W2, f32) @ np.asarray(al2, f32)).reshape(DH, 1),
            wr2=(np.asarray(W2, f32) @ np.asarray(ar2, f32)).reshape(DH, 1),
            wl3=(np.asarray(W3, f32) @ np.asarray(al3, f32)).reshape(DH, 1),
            wr3=(np.asarray(W3, f32) @ np.asarray(ar3, f32)).reshape(DH, 1),
        )
        in_maps.append(m)

    return dict(in_maps=in_maps, TOT=TOT, seg_meta=seg_meta, nch=nch)


def build_program(prep):
    import concourse.bacc as bacc
    import concourse.mybir as mybir
    import concourse.tile as tile
    from concourse import library_config

    f32 = mybir.dt.float32
    i16 = mybir.dt.int16
    AF = mybir.ActivationFunctionType
    OP = mybir.AluOpType
    TOT = prep["TOT"]
    seg_meta = prep["seg_meta"]
    maxl = int(os.environ.get("GAT_MAXL", "3"))
    noedge = os.environ.get("GAT_NOEDGE")
    nors = os.environ.get("GAT_NORS")
    nogather = os.environ.get("GAT_NOGATHER")
    nooex = os.environ.get("GAT_NOOEX")
    nomm = os.environ.get("GAT_NOMM")
    nostage = os.environ.get("GAT_NOSTAGE")
    nozmm = os.environ.get("GAT_NOZMM")
    noamm = os.environ.get("GAT_NOAMM")
    nopost = os.environ.get("GAT_NOPOST")
    maxseg = int(os.environ.get("GAT_MAXSEG", str(NSEG)))

    nc = bacc.Bacc("TRN2", target_bir_lowering=False, debug=False, num_devices=NCORES)

    hT = nc.dram_tensor("hT", [DIN, PC], f32, kind="ExternalInput")
    sidx_d = nc.dram_tensor("sidx", [P, TOT * 8], i16, kind="ExternalInput")
    dhi_d = nc.dram_tensor("dhi", [P, TOT * 8], i16, kind="ExternalInput")
    dlo_d = nc.dram_tensor("dlo", [P, TOT], f32, kind="ExternalInput")
    dstloc_d = nc.dram_tensor("dstloc", [P, TOT], f32, kind="ExternalInput")
    ex1_d = nc.dram_tensor("ex1", [P, TOT], f32, kind="ExternalInput")
    iota_d = nc.dram_tensor("iota_t", [P, P], f32, kind="ExternalInput")
    ident_d = nc.dram_tensor("ident_t", [P, P], f32, kind="ExternalInput")
    W1_d = nc.dram_tensor("W1", [DIN, DH], f32, kind="ExternalInput")
    W2_d = nc.dram_tensor("W2", [DH, DH], f32, kind="ExternalInput")
    W3_d = nc.dram_tensor("W3", [DH, DOUT], f32, kind="ExternalInput")
    wl2_d = nc.dram_tensor("wl2", [DH, 1], f32, kind="ExternalInput")
    wr2_d = nc.dram_tensor("wr2", [DH, 1], f32, kind="ExternalInput")
    wl3_d = nc.dram_tensor("wl3", [DH, 1], f32, kind="ExternalInput")
    wr3_d = nc.dram_tensor("wr3", [DH, 1], f32, kind="ExternalInput")
    out_d = nc.dram_tensor("out", [PC, DOUT], f32, kind="ExternalOutput")

    # (table_cols, gather_elem, el_col, agg_n, stage_cols)
    LAY = {
        1: dict(din=DIN, dn=DH, tc=128, elem=128, elcol=None, cst=132),
        2: dict(din=DH, dn=DH, tc=192, elem=192, elcol=128, cst=132),
        3: dict(din=DH, dn=DOUT, tc=128, elem=128, elcol=64, cst=68),
    }

    with tile.TileContext(nc) as tc:
        with (
            tc.tile_pool(name="sbP", bufs=1) as sbP,      # persistent-ish singles use bufs via tags
            tc.tile_pool(name="sbG", bufs=2) as sbG,      # streaming gather bufs
            tc.tile_pool(name="sbS", bufs=3) as sbS,      # small streaming tiles
            tc.tile_pool(name="psA", bufs=3, space="PSUM") as psA,
            tc.tile_pool(name="psB", bufs=4, space="PSUM") as psB,
            tc.tile_pool(name="dram", bufs=1, space="DRAM") as dram,
        ):
            nc.gpsimd.load_library(library_config.mlp)

            iota = sbP.tile([P, P], f32, tag="iota")
            nc.sync.dma_start(iota[:], iota_d[:])
            ident = sbP.tile([P, P], f32, tag="ident")
            nc.sync.dma_start(ident[:], ident_d[:])
            ones = sbP.tile([P, 1], f32, tag="ones")
            nc.gpsimd.memset(ones[:], 1.0)

            # weights in SBUF
            Wt = {}
            Wt[1] = [sbP.tile([P, DH], f32, tag=f"w1_{k}", name=f"w1_{k}") for k in range(2)]
            for k in range(2):
                nc.sync.dma_start(Wt[1][k][:], W1_d[k * P:(k + 1) * P, :])
            Wt[2] = [sbP.tile([P, DH], f32, tag="w2", name="w2")]
            nc.sync.dma_start(Wt[2][0][:], W2_d[:])
            Wt[3] = [sbP.tile([P, DOUT], f32, tag="w3", name="w3")]
            nc.sync.dma_start(Wt[3][0][:], W3_d[:])
            wv = {}
            for nm, d in (("wl2", wl2_d), ("wr2", wr2_d), ("wl3", wl3_d), ("wr3", wr3_d)):
                t = sbP.tile([P, 1], f32, tag=nm, name=nm)
                nc.sync.dma_start(t[:], d[:])
                wv[nm] = t

            # persistent x^T for layers 2/3 (written by previous layer's post)
            xT_buf = sbP.tile([P, PC], f32, tag="xT", name="xT")
            xT = {2: xT_buf, 3: xT_buf}

            for rep in range(int(os.environ.get("GAT_REPEAT", "1"))):
              # per-layer DRAM scratch (fresh per repeat: Shared tiles need one writer)
              tabs = {l: dram.tile([PC, LAY[l]["tc"]], f32, name=f"tab{l}_{rep}") for l in (1, 2, 3)}
              er_own = {l: dram.tile([PC], f32, name=f"er_own{l}_{rep}") for l in (2, 3)}
              er_full = {l: dram.tile([NPAD], f32, addr_space="Shared", name=f"er_full{l}_{rep}") for l in (2, 3)}
              rs_in = {(l, s): dram.tile([WPS * NCORES * P, LAY[l]["cst"]], f32,
                                         name=f"rs_in{l}_{s}_{rep}")
                       for l in (1, 2, 3) for s in range(NSEG)}
              rs_out = {(l, s): dram.tile([WPS * P, LAY[l]["cst"]], f32,
                                          name=f"rs_out{l}_{s}_{rep}")
                        for l in (1, 2, 3) for s in range(NSEG)}
              for l in (1, 2, 3):
                if l > maxl:
                    break
                L = LAY[l]
                din, dn, tcw, elem, elcol, cst = (
                    L["din"], L["dn"], L["tc"], L["elem"], L["elcol"], L["cst"])
                KT = din // P

                # ---- projection: build f table (+el col) and er_own ----
                er_stage = sbP.tile([P, WPC], f32, tag="er_stage", name="er_stage") if l >= 2 else None
                for t in range(WPC):
                    xts = []
                    if l == 1:
                        for k in range(KT):
                            xt = sbS.tile([P, P], f32, tag="hTk")
                            nc.sync.dma_start(
                                xt[:], hT[k * P:(k + 1) * P, t * P:(t + 1) * P])
                            xts.append(xt[:])
                    else:
                        xts = [xT[l][:, t * P:(t + 1) * P]]
                    f_ps = psA.tile([P, dn], f32, space="PSUM", tag="pbig")
                    for k in range(KT):
                        nc.tensor.matmul(f_ps[:], xts[k], Wt[l][k][:],
                                         start=(k == 0), stop=(k == KT - 1))
                    stage = sbS.tile([P, tcw], f32, tag="tstage")
                    nc.scalar.copy(stage[:, 0:dn], f_ps[:])
                    if l >= 2:
                        el_ps = psB.tile([P, 1], f32, space="PSUM", tag="pcol")
                        er_ps = psB.tile([P, 1], f32, space="PSUM", tag="pcol")
                        nc.tensor.matmul(el_ps[:], xts[0], wv[f"wl{l}"][:],
                                         start=True, stop=True)
                        nc.tensor.matmul(er_ps[:], xts[0], wv[f"wr{l}"][:],
                                         start=True, stop=True)
                        nc.vector.tensor_copy(stage[:, elcol:elcol + 1], el_ps[:])
                        nc.vector.tensor_copy(er_stage[:, t:t + 1], er_ps[:])
                    nc.sync.dma_start(tabs[l][t * P:(t + 1) * P, :], stage[:])

                if l >= 2:
                    # er_stage [128,49] -> flat [6272] via PE transpose
                    tr_ps = psA.tile([WPC, P], f32, space="PSUM", tag="pbig")
                    nc.tensor.transpose(tr_ps[:], er_stage[:], ident[:])
                    er_sb = sbS.tile([WPC, P], f32, tag="er_flat")
                    nc.vector.tensor_copy(er_sb[:], tr_ps[:])
                    nc.sync.dma_start(
                        er_own[l][:].rearrange("(t p) -> t p", p=P), er_sb[:])
                    nc.gpsimd.collective_compute(
                        "AllGather", mybir.AluOpType.bypass,
                        ins=[er_own[l][:]],
                        outs=[er_full[l][:]],
                        replica_groups=[list(range(NCORES))])

                # ---- edge phase ----
                for s in range(min(NSEG, maxseg)):
                    if noedge:
                        break
                    for (c0, c1, gwins) in seg_meta[s]:
                        nchg = c1 - c0
                        si = sbS.tile([P, nchg * 8], i16, tag="sidx")
                        nc.sync.dma_start(si[:], sidx_d[:, c0 * 8:c1 * 8])
                        G = sbG.tile([P, GMAX, elem], f32, tag=f"G{elem}")
                        if not nogather:
                            nc.gpsimd.dma_gather(
                                G[:, 0:nchg, :], tabs[l][:], si[:],
                                nchg * P, nchg * P, elem, single_packet=False)
                        else:
                            nc.vector.memset(G[:], 0.5)
                        dstl = sbS.tile([P, nchg], f32, tag="dstl")
                        nc.sync.dma_start(dstl[:], dstloc_d[:, c0:c1])
                        if l == 1:
                            exw = sbS.tile([P, GMAX], f32, tag="exw")
                            nc.sync.dma_start(exw[:, 0:nchg], ex1_d[:, c0:c1])
                            exv = exw[:].rearrange("p (c u) -> p c u", u=1)
                        else:
                            di = sbS.tile([P, nchg * 8], i16, tag="dhi")
                            nc.sync.dma_start(di[:], dhi_d[:, c0 * 8:c1 * 8])
                            erG = sbG.tile([P, GMAX, 64], f32, tag="erG")
                            nc.gpsimd.dma_gather(
                                erG[:, 0:nchg, :],
                                er_full[l][:].rearrange("(r k) -> r k", k=64),
                                di[:], nchg * P, nchg * P, 64,
                                single_packet=False)
                            dlo = sbS.tile([P, nchg], f32, tag="dlo")
                            nc.sync.dma_start(dlo[:], dlo_d[:, c0:c1])
                            ere = sbS.tile([P, GMAX], f32, tag="ere")
                            for ci in range(nchg):
                                scr = sbS.tile([P, 64], f32, tag="scr")
                                nc.vector.scalar_tensor_tensor(
                                    out=scr[:], in0=iota[:, 0:64],
                                    scalar=dlo[:, ci:ci + 1], in1=erG[:, ci, :],
                                    op0=OP.is_equal, op1=OP.mult,
                                    accum_out=ere[:, ci:ci + 1])
                            sc = sbS.tile([P, GMAX], f32, tag="sc")
                            nc.vector.tensor_tensor(
                                out=sc[:, 0:nchg],
                                in0=G[:, 0:nchg, elcol:elcol + 1].rearrange(
                                    "p c u -> p (c u)"),
                                in1=ere[:, 0:nchg], op=OP.add)
                            lr = sbS.tile([P, GMAX], f32, tag="lr")
                            nc.vector.scalar_tensor_tensor(
                                out=lr[:, 0:nchg], in0=sc[:, 0:nchg], scalar=0.2,
                                in1=sc[:, 0:nchg], op0=OP.mult, op1=OP.max)
                            exw = sbS.tile([P, GMAX], f32, tag="exw")
                            nc.scalar.activation(exw[:, 0:nchg], lr[:, 0:nchg], AF.Exp)
                            exv = exw[:].rearrange("p (c u) -> p c u", u=1)

                        for (w, stage_row, wc0, wc1) in gwins:
                            agg = psA.tile([P, dn], f32, space="PSUM", tag="pbig")
                            zps = psB.tile([P, 1], f32, space="PSUM", tag="pcol")
                            for ci in range(wc0, wc1):
                                oex = sbS.tile([P, P], f32, tag="oex")
                                if not nooex:
                                    nc.vector.tensor_scalar(
                                        out=oex[:], in0=iota[:],
                                        scalar1=dstl[:, ci:ci + 1],
                                        scalar2=exv[:, ci, 0:1],
                                        op0=OP.is_equal, op1=OP.mult)
                                else:
                                    nc.vector.memset(oex[:], 0.0)
                                if not nomm:
                                    if not noamm:
                                        nc.tensor.matmul(agg[:], oex[:], G[:, ci, 0:dn],
                                                         start=(ci == wc0), stop=(ci == wc1 - 1))
                                    if not nozmm:
                                        nc.tensor.matmul(zps[:], oex[:], ones[:],
                                                         start=(ci == wc0), stop=(ci == wc1 - 1))
                            if not nostage and not nomm and not noamm and not nozmm:
                                rstage = sbS.tile([P, cst], f32, tag="rstage")
                                nc.scalar.copy(rstage[:, 0:dn], agg[:])
                                nc.vector.tensor_copy(
                                    rstage[:, dn:cst], zps[:].to_broadcast([P, cst - dn]))
                                nc.sync.dma_start(
                                    rs_in[(l, s)][stage_row:stage_row + P, :], rstage[:])

                    if not nors:
                        nc.gpsimd.collective_compute(
                            "ReduceScatter", mybir.AluOpType.add,
                            ins=[rs_in[(l, s)][:]], outs=[rs_out[(l, s)][:]],
                            replica_groups=[list(range(NCORES))])

                # ---- post: normalize (+relu / log_softmax), build xT ----
                for s in range(min(NSEG, maxseg) if not nopost else 0):
                    for j in range(WPS):
                        rt = sbS.tile([P, cst], f32, tag="rpost")
                        nc.sync.dma_start(
                            rt[:], rs_out[(l, s)][j * P:(j + 1) * P, :])
                        zc = sbS.tile([P, 1], f32, tag="zc")
                        nc.vector.tensor_scalar(
                            out=zc[:], in0=rt[:, dn:dn + 1], scalar1=1e-9,
                            scalar2=None, op0=OP.max)
                        zrec = sbS.tile([P, 1], f32, tag="zrec")
                        nc.vector.reciprocal(zrec[:], zc[:])
                        if l < 3:
                            xw = sbS.tile([P, dn], f32, tag="xw")
                            nc.vector.tensor_scalar(
                                out=xw[:], in0=rt[:, 0:dn], scalar1=zrec[:],
                                scalar2=0.0, op0=OP.mult, op1=OP.max)
                            xtp = psA.tile([P, P], f32, space="PSUM", tag="pbig")
                            nc.tensor.transpose(xtp[:], xw[:], ident[:])
                            blk = (s * WPS + j) * P
                            nc.scalar.copy(xT[l + 1][:, blk:blk + P], xtp[:])
                        else:
                            xs = sbS.tile([P, DOUT], f32, tag="ls1")
                            nc.vector.tensor_scalar(
                                out=xs[:], in0=rt[:, 0:DOUT], scalar1=zrec[:],
                                scalar2=None, op0=OP.mult)
                            mx = sbS.tile([P, 1], f32, tag="mx")
                            nc.vector.tensor_reduce(
                                out=mx[:], in_=xs[:], op=OP.max,
                                axis=mybir.AxisListType.X)
                            xm = sbS.tile([P, DOUT], f32, tag="xm")
                            nc.vector.tensor_scalar(
                                out=xm[:], in0=xs[:], scalar1=mx[:],
                                scalar2=None, op0=OP.subtract)
                            ee = sbS.tile([P, DOUT], f32, tag="ee")
                            nc.scalar.activation(ee[:], xm[:], AF.Exp)
                            se = sbS.tile([P, 1], f32, tag="se")
                            nc.vector.tensor_reduce(
                                out=se[:], in_=ee[:], op=OP.add,
                                axis=mybir.AxisListType.X)
                            ls = sbS.tile([P, 1], f32, tag="lsum")
                            nc.scalar.activation(ls[:], se[:], AF.Ln)
                            fo = sbS.tile([P, DOUT], f32, tag="fout")
                            nc.vector.tensor_scalar(
                                out=fo[:], in0=xm[:], scalar1=ls[:],
                                scalar2=None, op0=OP.subtract)
                            blk = (s * WPS + j) * P
                            nc.sync.dma_start(out_d[blk:blk + P, :], fo[:])

    nc.compile()
    return nc


def kernel(**inputs):
    from concourse.bass_utils import run_bass_kernel_spmd

    prep = host_prep(**inputs)
    nc = build_program(prep)
    res = run_bass_kernel_spmd(nc, prep["in_maps"], core_ids=list(range(NCORES)))
    out = np.concatenate([res.results[c]["out"] for c in range(NCORES)], axis=0)
    return np.ascontiguousarray(out[:N]).astype(np.float32)



# revision 6
# speedup vs baseline: 54.7620x; 54.7620x over previous
"""3-layer GAT on 8 trn2 NeuronCores — instruction-count-minimized design.

This stack has a ~70us fixed cost per engine instruction, so the kernel is
built around a few fat gpsimd DMA ops per ~3K edges instead of per-chunk
one-hot matmuls:

  per layer: project x->f table (f|el|ones cols) in DRAM; per edge batch:
  dma_gather f[src] rows, compute ex=exp(leaky(el_src+er_dst)) with ~8 fat
  DVE ops (er picked from an AllGathered er_full via a 64-wide one-hot
  select), scale rows by ex, dma_scatter_add into a per-core [50176, C]
  DRAM accumulator (row = pi(dst)); one ReduceScatter delivers summed rows
  (agg | z) to the dst owner; post normalizes (relu / log_softmax).

Scatter-add correctness: duplicate dst rows within one scatter race across
DMA rings, so edges are split into rounds with unique dst per round; calls
are serialized by the accumulator WAW dependency. int16 scatter indices cap
at 32767 -> the accumulator is addressed in two 25088-row halves.

Node order is "rho-space": local node n=(t*128+p) lives at table row
rho=p*49+t, which makes every DRAM<->SBUF layout partition-contiguous and
lets layers 2/3 load x^T with a single 2-byte dma transpose.
"""

import os
import numpy as np

N, E, DIN, DH, DOUT = 50000, 800000, 256, 128, 64
NCORES = 8
PC = 6272            # nodes per core
NPAD = PC * NCORES   # 50176
WPC = 49             # 128-row windows per core
P = 128
HALF = NPAD // 2     # 25088 (< int16 max)
GMAX = 30            # chunks (128 slots) per batch
TC12, TC3 = 192, 128  # table/acc row widths (f32)


def _wrap16(idx, dtype=np.int16):
    """dma_gather/scatter index layout: [128, n/16]; idx j at [j%16+16k, j//16]."""
    n = len(idx)
    out = np.zeros((P, n // 16), dtype=dtype)
    out[:16, :] = idx.astype(dtype).reshape(-1, 16).T
    out[16:, :] = np.tile(out[:16, :], (7, 1))
    return out


def _leaky(x):
    return np.maximum(x, 0.2 * x)


def host_prep(h, src, dst, W1, al1, ar1, W2, al2, ar2, W3, al3, ar3):
    f32 = np.float32
    h = np.asarray(h, f32)
    src = np.asarray(src, np.int64)
    dst = np.asarray(dst, np.int64)

    hp = np.zeros((NPAD, DIN), f32)
    hp[:N] = h

    # rho-space: local node n=(t*128+p) -> table row rho=p*49+t
    nn = np.arange(PC)
    rho = (nn % P) * WPC + (nn // P)             # node -> row
    rho_inv = np.empty(PC, np.int64)
    rho_inv[rho] = nn                            # row -> node
    g_of_dst = (dst // PC) * PC + rho[dst % PC]  # pi(dst): acc row in [0,NPAD)

    # layer-1 edge scores on host (x == h)
    wl1 = (np.asarray(W1, f32) @ np.asarray(al1, f32)).astype(f32)
    wr1 = (np.asarray(W1, f32) @ np.asarray(ar1, f32)).astype(f32)
    el1 = hp @ wl1
    er1 = hp @ wr1
    ex1_edge = np.exp(_leaky(el1[src] + er1[dst])).astype(f32)

    core_of = src // PC

    # ---- shared batch schedule: (half, round) with unique dst per round ----
    per_core = []
    maxr = 0
    for c in range(NCORES):
        sel = np.nonzero(core_of == c)[0]
        g = g_of_dst[sel]
        half = (g >= HALF).astype(np.int64)
        order = np.argsort(g, kind="stable")
        gs = g[order]
        run_start = np.where(np.r_[True, gs[1:] != gs[:-1]],
                             np.arange(len(gs)), -1)
        rnd_sorted = np.arange(len(gs)) - np.maximum.accumulate(run_start)
        rnd = np.empty(len(gs), np.int64)
        rnd[order] = rnd_sorted
        maxr = max(maxr, int(rnd.max()) + 1)
        per_core.append((sel, g, half, rnd))
    per_core = [(sel, g, half, (rnd + g) % maxr)
                for (sel, g, half, rnd) in per_core]

    counts = np.zeros((NCORES, 2, maxr), np.int64)
    for c, (sel, g, half, rnd) in enumerate(per_core):
        np.add.at(counts[c], (half, rnd), 1)
    size_hr = counts.max(axis=0)                 # shared sizes
    nch_hr = np.maximum(1, -(-size_hr // P))     # chunks per (half, round)

    batches = []                                 # (half, c0, c1) chunk ranges
    chunk0_hr = np.zeros((2, maxr), np.int64)
    off = 0
    for hf in range(2):
        for r in range(maxr):
            ncj = int(nch_hr[hf, r])
            chunk0_hr[hf, r] = off
            done = 0
            sz = int(size_hr[hf, r])
            while done < ncj:
                take = min(GMAX, ncj - done)
                valid = max(1, min(sz - done * P, take * P))
                batches.append((hf, off + done, off + done + take, valid))
                done += take
            off += ncj
    TOT = off

    in_maps = []
    for c, (sel, g, half, rnd) in enumerate(per_core):
        e_src_row = rho[src[sel] % PC]           # gather row in own table
        e_ex1 = ex1_edge[sel]

        slot = np.zeros(len(sel), np.int64)
        for hf in range(2):
            m_h = half == hf
            for r in range(maxr):
                m = np.nonzero(m_h & (rnd == r))[0]
                if len(m):
                    slot[m] = chunk0_hr[hf, r] * P + np.arange(len(m))

        S = TOT * P
        sidx = np.zeros(S, np.int64)
        scat = np.zeros(S, np.int64)
        dhi = np.zeros(S, np.int64)
        dlo = np.zeros(S, f32)
        mskv = np.zeros(S, f32)
        exv = np.zeros(S, f32)

        sidx[slot] = e_src_row
        scat[slot] = g - half * HALF
        dhi[slot] = g >> 6
        dlo[slot] = (g & 63).astype(f32)
        mskv[slot] = 1.0
        exv[slot] = e_ex1

        dm = np.stack([dlo, mskv], axis=1)       # [S, 2]

        hp_c = hp[c * PC:(c + 1) * PC]
        W2e = np.concatenate(
            [np.asarray(W2, f32),
             (np.asarray(W2, f32) @ np.asarray(al2, f32))[:, None],
             (np.asarray(W2, f32) @ np.asarray(ar2, f32))[:, None]], axis=1)
        W3e = np.concatenate(
            [np.asarray(W3, f32),
             (np.asarray(W3, f32) @ np.asarray(al3, f32))[:, None],
             (np.asarray(W3, f32) @ np.asarray(ar3, f32))[:, None]], axis=1)

        m = dict(
            hT=np.ascontiguousarray(hp_c[rho_inv].T),            # [256, PC]
            sidx=_wrap16(sidx),
            scat=_wrap16(scat),
            dhi=_wrap16(dhi),
            dm=np.ascontiguousarray(
                dm.reshape(TOT, P, 2).transpose(1, 0, 2).reshape(P, TOT * 2)),
            ex1=np.ascontiguousarray(exv.reshape(TOT, P).T),
            iota=np.tile(np.arange(P, dtype=f32), (P, 1)),
            W1=np.asarray(W1, f32),
            W2e=W2e, W3e=W3e,
        )
        in_maps.append(m)

    return dict(in_maps=in_maps, TOT=TOT, batches=batches, rho=rho,
                rho_inv=rho_inv, g_of_dst=g_of_dst)


def build_program(prep):
    import concourse.bacc as bacc
    import concourse.mybir as mybir
    import concourse.tile as tile
    from concourse import library_config

    f32 = mybir.dt.float32
    bf16 = mybir.dt.bfloat16
    i16 = mybir.dt.int16
    AF = mybir.ActivationFunctionType
    OP = mybir.AluOpType
    TOT = prep["TOT"]
    batches = prep["batches"]
    maxl = int(os.environ.get("GAT_MAXL", "3"))
    noedge = os.environ.get("GAT_NOEDGE")
    nors = os.environ.get("GAT_NORS")
    nopost = os.environ.get("GAT_NOPOST")
    noscat = os.environ.get("GAT_NOSCAT")
    noscale = os.environ.get("GAT_NOSCALE")

    nc = bacc.Bacc("TRN2", target_bir_lowering=False, debug=False,
                   num_devices=NCORES)

    hT_d = nc.dram_tensor("hT", [DIN, PC], f32, kind="ExternalInput")
    sidx_d = nc.dram_tensor("sidx", [P, TOT * 8], i16, kind="ExternalInput")
    scat_d = nc.dram_tensor("scat", [P, TOT * 8], i16, kind="ExternalInput")
    dhi_d = nc.dram_tensor("dhi", [P, TOT * 8], i16, kind="ExternalInput")
    dm_d = nc.dram_tensor("dm", [P, TOT * 2], f32, kind="ExternalInput")
    ex1_d = nc.dram_tensor("ex1", [P, TOT], f32, kind="ExternalInput")
    iota_d = nc.dram_tensor("iota", [P, P], f32, kind="ExternalInput")
    W1_d = nc.dram_tensor("W1", [DIN, DH], f32, kind="ExternalInput")
    W2e_d = nc.dram_tensor("W2e", [DH, DH + 2], f32, kind="ExternalInput")
    W3e_d = nc.dram_tensor("W3e", [DH, DOUT + 2], f32, kind="ExternalInput")
    out_d = nc.dram_tensor("out", [P, WPC * DOUT], f32, kind="ExternalOutput")

    LAY = {
        1: dict(dn=DH, tc=TC12, elc=DH, onec=DH + 1),
        2: dict(dn=DH, tc=TC12, elc=DH, onec=DH + 1),
        3: dict(dn=DOUT, tc=TC3, elc=DOUT, onec=DOUT + 1),
    }

    with tile.TileContext(nc) as tc:
        with (
            tc.tile_pool(name="sbP", bufs=1) as sbP,
            tc.tile_pool(name="sbG", bufs=2) as sbG,
            tc.tile_pool(name="sbE", bufs=2) as sbE,
            tc.tile_pool(name="sbS", bufs=3) as sbS,
            tc.tile_pool(name="psA", bufs=4, space="PSUM") as psA,
            tc.tile_pool(name="dram", bufs=1, space="DRAM") as dram,
        ):
            nc.gpsimd.load_library(library_config.mlp)

            iota = sbP.tile([P, P], f32, tag="iota")
            nc.sync.dma_start(iota[:], iota_d[:])
            ones = sbP.tile([P, 1], f32, tag="ones")
            nc.gpsimd.memset(ones[:], 1.0)
            zero = sbP.tile([P, 2688], f32, tag="zero")
            nc.vector.memset(zero[:], 0.0)

            W1t = [sbP.tile([P, DH], f32, tag=f"w1_{k}", name=f"w1_{k}")
                   for k in range(2)]
            for k in range(2):
                nc.sync.dma_start(W1t[k][:], W1_d[k * P:(k + 1) * P, :])
            W2t = sbP.tile([P, DH + 2], bf16, tag="w2")
            nc.gpsimd.dma_start(out=W2t[:], in_=W2e_d[:])
            W3t = sbP.tile([P, DOUT + 2], bf16, tag="w3")
            nc.gpsimd.dma_start(out=W3t[:], in_=W3e_d[:])

            for rep in range(int(os.environ.get("GAT_REPEAT", "1"))):
                tabs = {l: dram.tile([PC, LAY[l]["tc"]], f32, name=f"tab{l}_{rep}")
                        for l in (1, 2, 3)}
                accs = {l: dram.tile([NPAD, LAY[l]["tc"]], f32, name=f"acc{l}_{rep}")
                        for l in (1, 2, 3)}
                rs_out = {l: dram.tile([PC, LAY[l]["tc"]], f32,
                                       name=f"rso{l}_{rep}") for l in (1, 2, 3)}
                er_own = {l: dram.tile([PC], f32, name=f"ero{l}_{rep}")
                          for l in (2, 3)}
                er_full = {l: dram.tile([NPAD], f32, addr_space="Shared",
                                        name=f"erf{l}_{rep}") for l in (2, 3)}
                x_dram = {l: dram.tile([PC, DH], bf16, name=f"x{l}_{rep}")
                          for l in (2, 3)}

                for l in (1, 2, 3):
                    if l > maxl:
                        break
                    L = LAY[l]
                    dn, tcw, elc, onec = L["dn"], L["tc"], L["elc"], L["onec"]

                    # ---- zero the accumulator (28 DMAs) ----
                    zrows = 1792                 # 14 rows per partition
                    zcols = zrows * tcw // P
                    for k in range(NPAD // zrows):
                        nc.sync.dma_start(
                            accs[l][k * zrows:(k + 1) * zrows, :].rearrange(
                                "(p r) c -> p (r c)", p=P),
                            zero[:, 0:zcols])

                    # ---- projection: tab rows [f | el | ones] ----
                    if l >= 2:
                        xT = sbP.tile([P, PC], bf16, tag="xT")
                        nc.sync.dma_start(xT[:], x_dram[l][:], transpose=True)
                        er_stage = sbP.tile([P, WPC], f32, tag="er_stage")
                    WB = 3
                    ncols = dn + (2 if l >= 2 else 0)
                    for t0 in range(0, WPC, WB):
                        tw = min(WB, WPC - t0)
                        ps = psA.tile([P, WB * ncols], f32, space="PSUM", tag="pproj")
                        for j in range(tw):
                            t = t0 + j
                            sl = ps[:, j * ncols:(j + 1) * ncols]
                            if l == 1:
                                for k in range(2):
                                    xt = sbS.tile([P, P], f32, tag="hTk")
                                    nc.sync.dma_start(
                                        xt[:], hT_d[k * P:(k + 1) * P,
                                                    t * P:(t + 1) * P])
                                    nc.tensor.matmul(sl, xt[:], W1t[k][:],
                                                     start=(k == 0), stop=(k == 1))
                            else:
                                Wt = W2t if l == 2 else W3t
                                nc.tensor.matmul(
                                    sl, xT[:, t * P:(t + 1) * P], Wt[:],
                                    start=True, stop=True)
                        stage = sbS.tile([P, WB, tcw], f32, tag="tstage")
                        ps3 = ps[:].rearrange("p (w c) -> p w c", c=ncols)
                        if l >= 2:
                            nc.scalar.copy(stage[:, 0:tw, 0:dn + 1],
                                           ps3[:, 0:tw, 0:dn + 1])
                            nc.vector.tensor_copy(
                                er_stage[:, t0:t0 + tw],
                                ps3[:, 0:tw, dn + 1:dn + 2].rearrange(
                                    "p w u -> p (w u)"))
                        else:
                            nc.scalar.copy(stage[:, 0:tw, 0:dn],
                                           ps3[:, 0:tw, 0:dn])
                        nc.vector.tensor_copy(
                            stage[:, 0:tw, onec:onec + 1],
                            ones[:].rearrange("p (u v) -> p u v", u=1).to_broadcast(
                                [P, tw, 1]))
                        nc.sync.dma_start(
                            tabs[l][t0 * P:(t0 + tw) * P, :].rearrange(
                                "(w p) c -> p w c", p=P),
                            stage[:, 0:tw, :])

                    if l >= 2:
                        nc.sync.dma_start(
                            er_own[l][:].rearrange("(w p) -> p w", p=P),
                            er_stage[:])
                        nc.gpsimd.collective_compute(
                            "AllGather", mybir.AluOpType.bypass,
                            ins=[er_own[l][:]], outs=[er_full[l][:]],
                            replica_groups=[list(range(NCORES))])

                    # ---- edge batches: gather -> ex -> scale -> scatter ----
                    for (hf, c0, c1, nv) in (batches if not noedge else []):
                        cb = c1 - c0
                        si = sbS.tile([P, GMAX * 8], i16, tag="si")
                        nc.sync.dma_start(si[:, 0:cb * 8], sidx_d[:, c0 * 8:c1 * 8])
                        G = sbG.tile([P, GMAX, tcw], f32, tag="G")
                        nc.gpsimd.dma_gather(
                            G[:, 0:cb, :], tabs[l][:], si[:, 0:cb * 8],
                            cb * P, cb * P, tcw, single_packet=False)
                        if l == 1:
                            exm = sbS.tile([P, GMAX], f32, tag="exm")
                            nc.sync.dma_start(exm[:, 0:cb], ex1_d[:, c0:c1])
                        else:
                            di = sbS.tile([P, GMAX * 8], i16, tag="di")
                            nc.sync.dma_start(di[:, 0:cb * 8],
                                              dhi_d[:, c0 * 8:c1 * 8])
                            erG = sbE.tile([P, GMAX, 64], f32, tag="erG")
                            nc.gpsimd.dma_gather(
                                erG[:, 0:cb, :],
                                er_full[l][:].rearrange("(r k) -> r k", k=64),
                                di[:, 0:cb * 8], cb * P, cb * P, 64,
                                single_packet=False)
                            dm = sbS.tile([P, GMAX, 2], f32, tag="dm")
                            nc.sync.dma_start(
                                dm[:, 0:cb, :].rearrange("p c k -> p (c k)"),
                                dm_d[:, c0 * 2:c1 * 2])
                            msk = sbE.tile([P, GMAX, 64], f32, tag="msk")
                            nc.vector.tensor_tensor(
                                out=msk[:, 0:cb, :],
                                in0=iota[:, 0:64].rearrange(
                                    "p (u d) -> p u d", u=1).to_broadcast([P, cb, 64]),
                                in1=dm[:, 0:cb, 0:1].to_broadcast([P, cb, 64]),
                                op=OP.is_equal)
                            nc.vector.tensor_tensor(
                                out=msk[:, 0:cb, :], in0=msk[:, 0:cb, :],
                                in1=erG[:, 0:cb, :], op=OP.mult)
                            ere = sbS.tile([P, GMAX], f32, tag="ere")
                            nc.vector.tensor_reduce(
                                out=ere[:, 0:cb], in_=msk[:, 0:cb, :],
                                op=OP.add, axis=mybir.AxisListType.X)
                            sc = sbS.tile([P, GMAX], f32, tag="sc")
                            nc.vector.tensor_tensor(
                                out=sc[:, 0:cb], in0=ere[:, 0:cb],
                                in1=G[:, 0:cb, elc:elc + 1].rearrange(
                                    "p c u -> p (c u)"), op=OP.add)
                            nc.vector.scalar_tensor_tensor(
                                out=sc[:, 0:cb], in0=sc[:, 0:cb], scalar=0.2,
                                in1=sc[:, 0:cb], op0=OP.mult, op1=OP.max)
                            exm = sbS.tile([P, GMAX], f32, tag="exm")
                            nc.scalar.activation(exm[:, 0:cb], sc[:, 0:cb], AF.Exp)
                            nc.vector.tensor_tensor(
                                out=exm[:, 0:cb], in0=exm[:, 0:cb],
                                in1=dm[:, 0:cb, 1:2].rearrange(
                                    "p c u -> p (c u)"), op=OP.mult)
                        if not noscale:
                            nc.vector.tensor_tensor(
                                out=G[:, 0:cb, :], in0=G[:, 0:cb, :],
                                in1=exm[:, 0:cb].rearrange("p (c v) -> p c v", v=1)
                                .to_broadcast([P, cb, tcw]), op=OP.mult)
                        if not noscat:
                            s2 = sbS.tile([P, GMAX * 8], i16, tag="s2")
                            nc.sync.dma_start(s2[:, 0:cb * 8],
                                              scat_d[:, c0 * 8:c1 * 8])
                            nc.gpsimd.dma_scatter_add(
                                accs[l][hf * HALF:(hf + 1) * HALF, :],
                                G[:, 0:cb, :], s2[:, 0:cb * 8],
                                nv, nv, tcw)

                    # ---- reduce-scatter + post ----
                    if not nors:
                        nc.gpsimd.collective_compute(
                            "ReduceScatter", mybir.AluOpType.add,
                            ins=[accs[l][:]], outs=[rs_out[l][:]],
                            replica_groups=[list(range(NCORES))])
                    if nopost:
                        continue
                    xp = sbP.tile([P, WPC, tcw], f32, tag="xp")
                    nc.sync.dma_start(
                        xp[:].rearrange("p t c -> p (t c)"),
                        rs_out[l][:].rearrange("(p t) c -> p (t c)", p=P))
                    zc = sbS.tile([P, WPC], f32, tag="zc")
                    nc.vector.tensor_scalar(
                        out=zc[:], in0=xp[:, :, onec:onec + 1].rearrange(
                            "p t u -> p (t u)"),
                        scalar1=1e-9, scalar2=None, op0=OP.max)
                    zr = sbS.tile([P, WPC], f32, tag="zr")
                    nc.vector.reciprocal(zr[:], zc[:])
                    if l < 3:
                        xb = sbP.tile([P, WPC, dn], bf16, tag="pq1")
                        nc.vector.tensor_tensor(
                            out=xb[:], in0=xp[:, :, 0:dn],
                            in1=zr[:].rearrange("p (t v) -> p t v", v=1).to_broadcast(
                                [P, WPC, dn]), op=OP.mult)
                        xb2 = sbP.tile([P, WPC, dn], bf16, tag="pq2")
                        nc.vector.tensor_scalar(
                            out=xb2[:], in0=xb[:], scalar1=0.0, scalar2=None,
                            op0=OP.max)
                        nc.sync.dma_start(
                            x_dram[l + 1][:].rearrange(
                                "(p t) c -> p (t c)", p=P),
                            xb2[:].rearrange("p t c -> p (t c)"))
                    else:
                        xs = sbP.tile([P, WPC, DOUT], f32, tag="pq1")
                        nc.vector.tensor_tensor(
                            out=xs[:], in0=xp[:, :, 0:DOUT],
                            in1=zr[:].rearrange("p (t v) -> p t v", v=1).to_broadcast(
                                [P, WPC, DOUT]), op=OP.mult)
                        mx = sbS.tile([P, WPC], f32, tag="mx")
                        nc.vector.tensor_reduce(
                            out=mx[:], in_=xs[:], op=OP.max,
                            axis=mybir.AxisListType.X)
                        xm = sbP.tile([P, WPC, DOUT], f32, tag="pq2")
                        nc.vector.tensor_tensor(
                            out=xm[:], in0=xs[:],
                            in1=mx[:].rearrange("p (t v) -> p t v", v=1).to_broadcast(
                                [P, WPC, DOUT]), op=OP.subtract)
                        ee = sbP.tile([P, WPC, DOUT], f32, tag="pq3")
                        nc.scalar.activation(ee[:], xm[:], AF.Exp)
                        se = sbS.tile([P, WPC], f32, tag="se")
                        nc.vector.tensor_reduce(
                            out=se[:], in_=ee[:], op=OP.add,
                            axis=mybir.AxisListType.X)
                        ls = sbS.tile([P, WPC], f32, tag="ls")
                        nc.scalar.activation(ls[:], se[:], AF.Ln)
                        fo = sbP.tile([P, WPC, DOUT], f32, tag="pq4")
                        nc.vector.tensor_tensor(
                            out=fo[:], in0=xm[:],
                            in1=ls[:].rearrange("p (t v) -> p t v", v=1).to_broadcast(
                                [P, WPC, DOUT]), op=OP.subtract)
                        nc.sync.dma_start(
                            out_d[:], fo[:].rearrange("p t c -> p (t c)"))

    nc.compile()
    return nc


def _unpermute(raw):
    """raw [P, WPC*DOUT] (row p*49+t) -> node-order [PC, DOUT]."""
    byrow = raw.reshape(P * WPC, DOUT)
    nn = np.arange(PC)
    return byrow[(nn % P) * WPC + (nn // P)]


def kernel(**inputs):
    from concourse.bass_utils import run_bass_kernel_spmd

    prep = host_prep(**inputs)
    nc = build_program(prep)
    res = run_bass_kernel_spmd(nc, prep["in_maps"], core_ids=list(range(NCORES)))
    full = np.concatenate(
        [_unpermute(np.asarray(res.results[c]["out"])) for c in range(NCORES)],
        axis=0)
    return np.ascontiguousarray(full[:N]).astype(np.float32)
